# revision 52
# baseline (speedup 1.0000x reference)
"""Trainium2 Bass kernel for single-head causal attention (nn_Head).

Problem: x [B=8, T=2048, E=1024] f32; Wq/Wk/Wv [1024, 128] f32.
  q,k,v = x @ W*;  A = softmax(causal(q k^T / sqrt(H)));  out = A v.

Sharding: data-parallel over batch B — one batch element per NeuronCore
(8 cores), weights replicated. No collectives; outputs gathered host-side.

Per-core algorithm (T=2048, E=1024, H=128), v7:
  - x is converted f32->bf16 (DVE/ACT alternating) then PE-transposed in
    bf16 at 1 cyc/row; transpose PSUM evacuations run at 2x accel.
  - qT/kT/v/es are bf16: the attention matmuls (S = k^T q, O^T = v^T es)
    run at 1 cyc/row, the causal trapezoid is exact at 128 granularity,
    and the diagonal mask is one 128-wide affine_select per diag chunk.
  - Denominator: es accumulated into a bf16 dacc tile on DVE, one fold
    matmul per t-block, e0-matmul transpose trick, reciprocal on DVE.
  - Epilogue: PE-transpose of O^T, normalize into two independent halves
    (so the two out-DMAs can fire as soon as their half is scaled).
  - Emission interleaves attention of t-block n-1 with conv+transpose+
    proj of block n; block-0 x DMA is issued before the weight DMAs on
    the same SP queue (weights aren't needed until the first projection).
  - HW-measured breakdown (repeat-delta slope): x-load pipeline ~38us,
    +projections ~40us, +attention/epilogue ~6us. The projection stage is
    LDWEIGHTS-serialization-bound on HW (full-128-row stationaries are
    never pulled ahead by the PE reorder window).
"""

import numpy as np

import concourse.bass as bass
import concourse.mybir as mybir
import concourse.tile as tile
from concourse import bacc
from concourse import bass_utils
from concourse.masks import make_identity

F32 = mybir.dt.float32
F32R = mybir.dt.float32r
BF16 = mybir.dt.bfloat16
AF = mybir.ActivationFunctionType

B, T, E, H = 8, 2048, 1024, 128
P = 128                 # partitions
NE = E // P             # 8 e-chunks
NT = T // P             # 16 t-tiles
TBW = 512               # t-block width
NTB = T // TBW          # 4 t-blocks
SCALE = float(H) ** -0.5

DENOM_MODE = "dve"      # "pe" (ones-matmul per chunk) | "dve" (dacc on DVE)
import os as _os0
PIPE = int(_os0.environ.get("K_PIPE", "3"))  # PV trails S/exp by this many
import os as _os
PROJ_EVAC = _os.environ.get("K_PROJ_EVAC", "act")    # act | pool | dve
TP_COPY_B = _os.environ.get("K_TP_COPY_B", "act")    # engine for odd tij
EPI_MUL_B = _os.environ.get("K_EPI_MUL_B", "dve")    # engine for j=2,3 muls
CONV_ENG = _os.environ.get("K_CONV_ENG", "dve_act")  # pool|pool_dve|pool_act|dve|dve_act
TRANS = _os.environ.get("K_TRANS", "bf16")           # hostbf16 | bf16 | f32
XQ = int(_os.environ.get("K_XQ", "1"))               # x DMA queues (1|2)
XB = _os.environ.get("K_XB", "tij2")                 # x DMA granularity


def _eng(nc, name):
    return {"act": None, "pool": nc.gpsimd, "dve": nc.vector}[name]


def emit_const_prologue(nc, tc, persist):
    ident = persist.tile([P, P], F32)
    make_identity(nc, ident)
    ident_bf = persist.tile([P, P], BF16)
    nc.vector.tensor_copy(ident_bf, ident)

    ones_f = persist.tile([P, 1], F32)
    nc.vector.memset(ones_f, 1.0)
    ones_bf = persist.tile([P, 1], BF16)
    nc.vector.tensor_copy(ones_bf, ones_f)
    # e0: unit vector selecting row 0; d_sb: staging for the denominator row
    # (rows 1..127 stay zero forever, so no per-iteration memset is needed)
    e0_f = persist.tile([P, 1], F32)
    nc.vector.memset(e0_f, 0.0)
    nc.vector.memset(e0_f[0:1, :], 1.0)
    d_sb = persist.tile([P, TBW], F32)
    nc.gpsimd.memset(d_sb, 0.0)
    return (ident, ident_bf), ones_bf, e0_f, d_sb


def emit_weights(nc, persist, wq_d, wk_d, wv_d):
    """Weight DMAs go on the same SP queue as x, so emit them AFTER block-0's
    x tiles: proj-q can't start before block-0 is transposed anyway, and x
    going first shrinks the startup PE-idle gap by ~6us."""
    w_f = []
    for name, wd in (("wq", wq_d), ("wk", wk_d), ("wv", wv_d)):
        wt = persist.tile([P, NE, H], F32, name=f"{name}_f")
        nc.sync.dma_start(out=wt, in_=wd.rearrange("(ec p) h -> p ec h", p=P))
        wb = persist.tile([P, NE, H], BF16, name=f"{name}_bf")
        nc.vector.tensor_copy(wb, wt)
        w_f.append(wb)
    return w_f


def make_pools(tc, stack):
    names = [("xstage", dict(bufs=2)),
             ("xbf", dict(bufs=2)),
             ("vstage", dict(bufs=2)),
             ("mm_ps", dict(bufs=3, space="PSUM")),
             ("s_ps", dict(bufs=3, space="PSUM")),
             ("o_ps", dict(bufs=1, space="PSUM")),
             ("d_ps", dict(bufs=1, space="PSUM")),
             ("es_pool", dict(bufs=6)),
             ("dacc_pool", dict(bufs=2)),
             ("epi", dict(bufs=2))]
    return {n: stack.enter_context(tc.tile_pool(name=n, **kw))
            for n, kw in names}


def emit_body(nc, tc, pools, idents, ones_bf, e0_f, d_sb, w_f, res,
              x_d, out_d, stages="all", emit_weights_cb=None):
    ident, identq = idents[0], (idents[1],)
    (xT, qT, kT, v_r, x_g, xb) = res
    vstage_pool = pools["vstage"]
    mm_ps, s_ps, o_ps, d_ps = (pools["mm_ps"], pools["s_ps"],
                               pools["o_ps"], pools["d_ps"])
    es_pool, dacc_pool, epi = (pools["es_pool"], pools["dacc_pool"],
                               pools["epi"])
    if True:

        pend = []
        deferred = []

        def tick_deferred(flush=False):
            for item in list(deferred):
                item[0] -= 1
                if flush or item[0] <= 0:
                    item[1]()
                    deferred.remove(item)

        def emit_epilogue(tb, o_t, d_t, dacc):
            if DENOM_MODE == "dve":
                nc.tensor.matmul(d_t, ones_bf, dacc, start=True, stop=True)
            # stage denominator row into d_sb row 0, transpose-replicate via
            # K=128 matmuls against e0 (rows 1.. of d_sb are always zero)
            nc.scalar.copy(out=d_sb[0:1, :], in_=d_t)
            # dtp lives in d_ps: d_t is dead once d_sb is staged, so the
            # ring slot chains naturally and mm_ps stays free for the next
            # block's transposes/projections
            dtp = d_ps.tile([P, TBW], F32, name="dtp", tag="d")
            for j in range(4):
                nc.tensor.matmul(
                    dtp[:, j:j + 1], d_sb[:, j * P:(j + 1) * P], e0_f,
                    start=True, stop=True,
                )
            recip = epi.tile([P, 4], F32, name="recip")
            nc.vector.reciprocal(recip, dtp[:, 0:4])
            oT = epi.tile([P, TBW], BF16, name="oT")
            nc.vector.tensor_copy(oT, o_t)
            # otp lives in o_ps: o_t is dead once oT is copied out
            otp = o_ps.tile([P, TBW], BF16, name="otp", tag="o")
            for j in range(4):
                nc.tensor.transpose(
                    otp[:, j * P:(j + 1) * P],
                    oT[:, j * P:(j + 1) * P],
                    identq[0],
                )
            # two independent halves (DVE: j=0,1; Pool: j=2,3) so the
            # normalize chains run in parallel and each half's out-DMA can
            # start as soon as its own muls finish
            o_hlf = [epi.tile([P, 2, P], F32, name=f"o_out{h}")
                     for h in range(2)]
            for j in range(4):
                eng = nc.vector if j < 2 else _eng(nc, EPI_MUL_B)
                eng.tensor_scalar_mul(
                    out=o_hlf[j // 2][:, j % 2, :],
                    in0=otp[:, j * P:(j + 1) * P],
                    scalar1=recip[:, j:j + 1],
                )

            def do_dma():
                for h in range(2):
                    nc.scalar.dma_start(
                        out=out_d[tb * TBW + h * 2 * P:
                                  tb * TBW + (h + 1) * 2 * P, :].rearrange(
                            "(j p) h -> p j h", p=P),
                        in_=o_hlf[h],
                    )
            deferred.append([3, do_dma])

        def pop_pv():
            tb, si, off, es, first, last, o_t, d_t, dacc = pend.pop(0)
            if DENOM_MODE == "dve":
                # deferred to pop time so the DVE queue head never waits on
                # the exp/mask chain (es is certainly ready by now)
                if first:
                    nc.vector.tensor_copy(dacc, es)
                else:
                    nc.vector.tensor_add(dacc[:, off:], dacc[:, off:],
                                         es[:, off:])
            nc.tensor.matmul(o_t[:, off:], v_r[:, si, :], es[:, off:],
                             start=first, stop=last, skip_group_check=True)
            if DENOM_MODE == "pe":
                nc.tensor.matmul(d_t[:, off:], ones_bf, es[:, off:],
                                 start=first, stop=last,
                                 skip_group_check=True)
            if last:
                emit_epilogue(tb, o_t, d_t, dacc)

        # attention chunk state (carried across sections)
        attn_state = {}

        def emit_attn_chunk(tb, si):
            n_sc = (tb + 1) * (TBW // P)
            if si == 0:
                attn_state["o_t"] = o_ps.tile([P, TBW], F32, name="o_t", tag="o")
                attn_state["d_t"] = d_ps.tile([1, TBW], F32, name="d_t", tag="d")
                attn_state["dacc"] = (
                    dacc_pool.tile([P, TBW], BF16, name="dacc")
                    if DENOM_MODE == "dve" else None)
                attn_state["spair"] = None
            o_t, d_t, dacc = (attn_state["o_t"], attn_state["d_t"],
                              attn_state["dacc"])
            diag = si >= 4 * tb
            off = 0 if not diag else (si - 4 * tb) * P
            s_t = s_ps.tile([P, TBW], F32, name="s_t")
            nc.tensor.matmul(
                s_t[:, off:], kT[:, si * P:(si + 1) * P],
                qT[:, tb * TBW + off:(tb + 1) * TBW],
                start=True, stop=True,
            )
            es = es_pool.tile([P, TBW], BF16, name="es")
            nc.scalar.activation(out=es[:, off:], in_=s_t[:, off:],
                                 func=AF.Exp, scale=SCALE)
            if diag:
                # zero entries above the diagonal; only the leading 128
                # columns of the chunk straddle it
                nc.gpsimd.affine_select(
                    out=es[:, off:off + P], in_=es[:, off:off + P],
                    compare_op=mybir.AluOpType.is_ge,
                    fill=0.0, base=0,
                    pattern=[[1, P]], channel_multiplier=-1,
                )
            pend.append((tb, si, off, es, si == 0, si == n_sc - 1,
                         o_t, d_t, dacc))
            while len(pend) > PIPE:
                pop_pv()
            tick_deferred()

        def emit_x_dma(n):
            buf = x_g[n % 2]
            if XB in ("block", "block2"):
                # one 2MB DMA per block: amortizes the per-DMA completion
                # latency (the dominant fixed cost on HW) over 512 rows.
                # block2: alternate the SP HWDGE ring and the gpsimd SWDGE
                # ring so the two drain in parallel.
                eng = nc.sync if (XB == "block" or n % 2 == 0) else nc.gpsimd
                eng.dma_start(
                    out=buf,
                    in_=x_d[n * 4 * P:(n + 1) * 4 * P, :].rearrange(
                        "(tij p) e -> p tij e", p=P))
                return
            for tij in range(4):
                r0 = (n * 4 + tij) * P
                if XB == "tij2":
                    # alternate SP HWDGE ring / gpsimd SWDGE ring per tile
                    eng = nc.sync if tij % 2 == 0 else nc.gpsimd
                else:
                    eng = nc.sync if (XQ == 1 or tij % 2 == 0) else nc.scalar
                if n == 0:
                    # split block-0 loads so the first convert/transpose can
                    # start one half-tile (~0.7us) earlier
                    for hf in range(2):
                        cs = hf * (E // 2)
                        eng.dma_start(
                            out=buf[:, tij, cs:cs + E // 2],
                            in_=x_d[r0:r0 + P, cs:cs + E // 2])
                else:
                    eng.dma_start(out=buf[:, tij, :], in_=x_d[r0:r0 + P, :])

        def make_stageA(n):
            """Convert + transpose + projections + v-transpose emitters."""
            buf = x_g[n % 2]
            xbuf = xb[n % 2]
            emitters = []

            def conv_emitter(tij, half):
                # f32 -> bf16 ahead of the transpose: PE transposes then run
                # at 1 cyc/row (vs 2 for f32) and evacuations at 2x accel.
                def emit():
                    dst = xbuf[:, tij, half * 4 * P:(half + 1) * 4 * P]
                    src = buf[:, tij, half * 4 * P:(half + 1) * 4 * P]
                    k = (tij * 2 + half) % 2
                    if CONV_ENG == "pool":
                        nc.gpsimd.tensor_copy(dst, src)
                    elif CONV_ENG == "pool_dve":
                        (nc.gpsimd if k == 0 else nc.vector).tensor_copy(
                            dst, src)
                    elif CONV_ENG == "pool_act":
                        if k == 0:
                            nc.gpsimd.tensor_copy(dst, src)
                        else:
                            nc.scalar.copy(out=dst, in_=src)
                    elif CONV_ENG == "dve_act":
                        if k == 0:
                            nc.vector.tensor_copy(dst, src)
                        else:
                            nc.scalar.copy(out=dst, in_=src)
                    else:
                        nc.vector.tensor_copy(dst, src)
                return emit

            def tp_emitter(tij, half):
                def emit():
                    if TRANS == "f32":
                        tp = mm_ps.tile([P, 4 * P], F32, name="tp", tag="mm")
                        for j in range(4):
                            ec = 4 * half + j
                            nc.tensor.transpose(
                                tp[:, j * P:(j + 1) * P],
                                buf[:, tij, ec * P:(ec + 1) * P],
                                ident,
                            )
                    else:
                        tp = mm_ps.tile([P, 4 * P], BF16, name="tp", tag="mm")
                        for j in range(4):
                            ec = 4 * half + j
                            nc.tensor.transpose(
                                tp[:, j * P:(j + 1) * P],
                                xbuf[:, tij, ec * P:(ec + 1) * P],
                                identq[0],
                            )
                    dst = xT[:, 4 * half:4 * half + 4,
                             n * TBW + tij * P:n * TBW + (tij + 1) * P]
                    src = tp.rearrange("p (a b) -> p a b", b=P)
                    if tij % 2 == 0:
                        nc.vector.tensor_copy(dst, src)
                    elif TP_COPY_B == "act":
                        nc.scalar.copy(out=dst, in_=src)
                    else:
                        _eng(nc, TP_COPY_B).tensor_copy(dst, src)
                return emit

            for tij in range(4):
                for half in range(2):
                    if TRANS == "bf16":
                        emitters.append(conv_emitter(tij, half))
                    emitters.append(tp_emitter(tij, half))

            if stages == "xonly":
                return emitters

            vstage = vstage_pool.tile([P, TBW], BF16, name="vstage")

            def proj_emitter(w, dst_slice):
                def emit():
                    pt = mm_ps.tile([P, TBW], F32, name="pt", tag="mm")
                    for ec in range(NE):
                        nc.tensor.matmul(
                            pt, w[:, ec, :],
                            xT[:, ec, n * TBW:(n + 1) * TBW],
                            start=(ec == 0), stop=(ec == NE - 1),
                        )
                    if PROJ_EVAC == "act":
                        nc.scalar.copy(out=dst_slice, in_=pt)
                    else:
                        _eng(nc, PROJ_EVAC).tensor_copy(dst_slice, pt)
                return emit

            emitters.append(proj_emitter(wq_f, qT[:, n * TBW:(n + 1) * TBW]))
            emitters.append(proj_emitter(wk_f, kT[:, n * TBW:(n + 1) * TBW]))
            emitters.append(proj_emitter(wv_f, vstage))

            def vtrans_emitter():
                vp = mm_ps.tile([P, 4 * P], BF16, name="vp", tag="mm")
                ident_bf = identq[0]
                for j in range(4):
                    nc.tensor.transpose(
                        vp[:, j * P:(j + 1) * P],
                        vstage[:, j * P:(j + 1) * P],
                        ident_bf,
                    )
                nc.vector.tensor_copy(
                    v_r[:, n * 4:(n + 1) * 4, :].rearrange("p a b -> p (a b)"),
                    vp)

            emitters.append(vtrans_emitter)
            return emitters

        # ---- main interleaved schedule ----
        emit_x_dma(0)
        if emit_weights_cb is not None:
            w_f = emit_weights_cb()
        wq_f, wk_f, wv_f = w_f
        for n in range(NTB):
            if n + 1 < NTB:
                emit_x_dma(n + 1)
            stageA = make_stageA(n)
            if stages in ("xonly", "xproj") or n == 0:
                chunks = []
            else:
                tb = n - 1
                chunks = [(tb, si) for si in range((tb + 1) * 4)]
            ci = 0
            for i, emit in enumerate(stageA):
                emit()
                want = (i + 1) * len(chunks) // len(stageA)
                while ci < want:
                    emit_attn_chunk(*chunks[ci])
                    ci += 1
        if stages not in ("xonly", "xproj"):
            tb = NTB - 1
            for si in range((tb + 1) * 4):
                emit_attn_chunk(tb, si)
            while pend:
                pop_pv()
            tick_deferred(flush=True)


_CACHED = {}


def build_program(repeat: int = 1, stages: str = "all"):
    key = (repeat, stages)
    if key in _CACHED:
        return _CACHED[key]
    nc = bacc.Bacc("TRN2", target_bir_lowering=False, debug=False,
                   num_devices=B)
    x_d = nc.dram_tensor("x", [T, E],
                         BF16 if TRANS == "hostbf16" else F32,
                         kind="ExternalInput").ap()
    wq_d = nc.dram_tensor("Wq", [E, H], F32, kind="ExternalInput").ap()
    wk_d = nc.dram_tensor("Wk", [E, H], F32, kind="ExternalInput").ap()
    wv_d = nc.dram_tensor("Wv", [E, H], F32, kind="ExternalInput").ap()
    out_d = nc.dram_tensor("out", [T, H], F32, kind="ExternalOutput").ap()

    import contextlib
    with tile.TileContext(nc) as tc:
        with contextlib.ExitStack() as stack:
            persist = stack.enter_context(tc.tile_pool(name="persist", bufs=1))
            idents, ones_bf, e0_f, d_sb = emit_const_prologue(nc, tc, persist)
            pools = make_pools(tc, stack)
            xT = persist.tile([P, NE, T], BF16)     # [e_local, ec, t]
            qT = persist.tile([P, T], BF16)         # [h, t]
            kT = persist.tile([P, T], BF16)         # [h, s]
            v_r = persist.tile([P, NT, H], BF16)    # [s_local, sc, h]
            x_g = [pools["xstage"].tile(
                       [P, 4, E], BF16 if TRANS == "hostbf16" else F32,
                       name="x_g")
                   for _ in range(2)]
            xb = ([pools["xbf"].tile([P, 4, E], BF16, name="xb")
                   for _ in range(2)] if TRANS == "bf16" else x_g)
            res = (xT, qT, kT, v_r, x_g, xb)
            if repeat > 1:
                w_f = emit_weights(nc, persist, wq_d, wk_d, wv_d)
                with tc.For_i(0, repeat, 1):
                    emit_body(nc, tc, pools, idents, ones_bf, e0_f, d_sb,
                              w_f, res, x_d, out_d, stages=stages)
            else:
                emit_body(nc, tc, pools, idents, ones_bf, e0_f, d_sb,
                          None, res, x_d, out_d, stages=stages,
                          emit_weights_cb=lambda: emit_weights(
                              nc, persist, wq_d, wk_d, wv_d))
    nc.compile()
    _CACHED[key] = nc
    return nc


def prep_x(x):
    """Host-side cast of x to the kernel's DRAM dtype (bf16 halves the
    x DMA bytes; the kernel would round to bf16 on-chip anyway)."""
    if TRANS == "hostbf16":
        import ml_dtypes
        return np.ascontiguousarray(np.asarray(x).astype(ml_dtypes.bfloat16))
    return np.ascontiguousarray(np.asarray(x, dtype=np.float32))


def kernel(x, Wk, Wq, Wv):
    x = prep_x(x)
    Wk = np.ascontiguousarray(np.asarray(Wk, dtype=np.float32))
    Wq = np.ascontiguousarray(np.asarray(Wq, dtype=np.float32))
    Wv = np.ascontiguousarray(np.asarray(Wv, dtype=np.float32))
    assert x.shape == (B, T, E), x.shape

    nc = build_program()
    in_maps = [
        {"x": np.ascontiguousarray(x[c]), "Wq": Wq, "Wk": Wk, "Wv": Wv}
        for c in range(B)
    ]
    res = bass_utils.run_bass_kernel_spmd(nc, in_maps, core_ids=list(range(B)))
    return np.stack([res.results[c]["out"] for c in range(B)], axis=0)


if __name__ == "__main__":
    rng = np.random.default_rng(0)
    x = rng.standard_normal((B, T, E), dtype=np.float32)
    wq = (rng.standard_normal((E, H), dtype=np.float32) / np.sqrt(E)).astype(np.float32)
    wk = (rng.standard_normal((E, H), dtype=np.float32) / np.sqrt(E)).astype(np.float32)
    wv = (rng.standard_normal((E, H), dtype=np.float32) / np.sqrt(E)).astype(np.float32)
    out = kernel(x, wk, wq, wv)
    print("out", out.shape, out.dtype, float(np.abs(out).max()))



# revision 54
# speedup vs baseline: 1.1416x; 1.1416x over previous
"""Trainium2 Bass kernel for single-head causal attention (nn_Head).

Problem: x [B=8, T=2048, E=1024] f32; Wq/Wk/Wv [1024, 128] f32.
  q,k,v = x @ W*;  A = softmax(causal(q k^T / sqrt(H)));  out = A v.

Sharding: data-parallel over batch B — one batch element per NeuronCore
(8 cores), weights replicated. No collectives; outputs gathered host-side.

Per-core algorithm (T=2048, E=1024, H=128), v7:
  - x is converted f32->bf16 (DVE/ACT alternating) then PE-transposed in
    bf16 at 1 cyc/row; transpose PSUM evacuations run at 2x accel.
  - qT/kT/v/es are bf16: the attention matmuls (S = k^T q, O^T = v^T es)
    run at 1 cyc/row, the causal trapezoid is exact at 128 granularity,
    and the diagonal mask is one 128-wide affine_select per diag chunk.
  - Denominator: es accumulated into a bf16 dacc tile on DVE, one fold
    matmul per t-block, e0-matmul transpose trick, reciprocal on DVE.
  - Epilogue: PE-transpose of O^T, normalize into two independent halves
    (so the two out-DMAs can fire as soon as their half is scaled).
  - Emission interleaves attention of t-block n-1 with conv+transpose+
    proj of block n; block-0 x DMA is issued before the weight DMAs on
    the same SP queue (weights aren't needed until the first projection).
  - HW-measured breakdown (repeat-delta slope): x-load pipeline ~38us,
    +projections ~40us, +attention/epilogue ~6us. The projection stage is
    LDWEIGHTS-serialization-bound on HW (full-128-row stationaries are
    never pulled ahead by the PE reorder window).
"""

import numpy as np

import concourse.bass as bass
import concourse.mybir as mybir
import concourse.tile as tile
from concourse import bacc
from concourse import bass_utils
from concourse.masks import make_identity

F32 = mybir.dt.float32
F32R = mybir.dt.float32r
BF16 = mybir.dt.bfloat16
AF = mybir.ActivationFunctionType

B, T, E, H = 8, 2048, 1024, 128
P = 128                 # partitions
NE = E // P             # 8 e-chunks
NT = T // P             # 16 t-tiles
TBW = 512               # t-block width
NTB = T // TBW          # 4 t-blocks
SCALE = float(H) ** -0.5

DENOM_MODE = "dve"      # "pe" (ones-matmul per chunk) | "dve" (dacc on DVE)
import os as _os0
PIPE = int(_os0.environ.get("K_PIPE", "3"))  # PV trails S/exp by this many
import os as _os
PROJ_EVAC = _os.environ.get("K_PROJ_EVAC", "act")    # act | pool | dve
TP_COPY_B = _os.environ.get("K_TP_COPY_B", "act")    # engine for odd tij
EPI_MUL_B = _os.environ.get("K_EPI_MUL_B", "dve")    # engine for j=2,3 muls
CONV_ENG = _os.environ.get("K_CONV_ENG", "dve_act")  # pool|pool_dve|pool_act|dve|dve_act
TRANS = _os.environ.get("K_TRANS", "bf16")           # hostbf16 | bf16 | f32
XQ = int(_os.environ.get("K_XQ", "1"))               # x DMA queues (1|2)
XB = _os.environ.get("K_XB", "tij")                 # x DMA granularity


def _eng(nc, name):
    return {"act": None, "pool": nc.gpsimd, "dve": nc.vector}[name]


def emit_const_prologue(nc, tc, persist):
    ident = persist.tile([P, P], F32)
    make_identity(nc, ident)
    ident_bf = persist.tile([P, P], BF16)
    nc.vector.tensor_copy(ident_bf, ident)

    ones_f = persist.tile([P, 1], F32)
    nc.vector.memset(ones_f, 1.0)
    ones_bf = persist.tile([P, 1], BF16)
    nc.vector.tensor_copy(ones_bf, ones_f)
    # e0: unit vector selecting row 0; d_sb: staging for the denominator row
    # (rows 1..127 stay zero forever, so no per-iteration memset is needed)
    e0_f = persist.tile([P, 1], F32)
    nc.vector.memset(e0_f, 0.0)
    nc.vector.memset(e0_f[0:1, :], 1.0)
    d_sb = persist.tile([P, TBW], F32)
    nc.gpsimd.memset(d_sb, 0.0)
    return (ident, ident_bf), ones_bf, e0_f, d_sb


def emit_weights(nc, persist, wq_d, wk_d, wv_d):
    """Weight DMAs go on the same SP queue as x, so emit them AFTER block-0's
    x tiles: proj-q can't start before block-0 is transposed anyway, and x
    going first shrinks the startup PE-idle gap by ~6us."""
    w_f = []
    for name, wd in (("wq", wq_d), ("wk", wk_d), ("wv", wv_d)):
        wt = persist.tile([P, NE, H], F32, name=f"{name}_f")
        nc.sync.dma_start(out=wt, in_=wd.rearrange("(ec p) h -> p ec h", p=P))
        wb = persist.tile([P, NE, H], BF16, name=f"{name}_bf")
        nc.vector.tensor_copy(wb, wt)
        w_f.append(wb)
    return w_f


def make_pools(tc, stack):
    names = [("xstage", dict(bufs=2)),
             ("xbf", dict(bufs=2)),
             ("vstage", dict(bufs=2)),
             ("mm_ps", dict(bufs=3, space="PSUM")),
             ("s_ps", dict(bufs=3, space="PSUM")),
             ("o_ps", dict(bufs=1, space="PSUM")),
             ("d_ps", dict(bufs=1, space="PSUM")),
             ("es_pool", dict(bufs=6)),
             ("dacc_pool", dict(bufs=2)),
             ("epi", dict(bufs=2))]
    return {n: stack.enter_context(tc.tile_pool(name=n, **kw))
            for n, kw in names}


def emit_body(nc, tc, pools, idents, ones_bf, e0_f, d_sb, w_f, res,
              x_d, out_d, stages="all", emit_weights_cb=None):
    ident, identq = idents[0], (idents[1],)
    (xT, qT, kT, v_r, x_g, xb) = res
    vstage_pool = pools["vstage"]
    mm_ps, s_ps, o_ps, d_ps = (pools["mm_ps"], pools["s_ps"],
                               pools["o_ps"], pools["d_ps"])
    es_pool, dacc_pool, epi = (pools["es_pool"], pools["dacc_pool"],
                               pools["epi"])
    if True:

        pend = []
        deferred = []

        def tick_deferred(flush=False):
            for item in list(deferred):
                item[0] -= 1
                if flush or item[0] <= 0:
                    item[1]()
                    deferred.remove(item)

        def emit_epilogue(tb, o_t, d_t, dacc):
            if DENOM_MODE == "dve":
                nc.tensor.matmul(d_t, ones_bf, dacc, start=True, stop=True)
            # stage denominator row into d_sb row 0, transpose-replicate via
            # K=128 matmuls against e0 (rows 1.. of d_sb are always zero)
            nc.scalar.copy(out=d_sb[0:1, :], in_=d_t)
            dtp = mm_ps.tile([P, TBW], F32, name="dtp", tag="mm")
            for j in range(4):
                nc.tensor.matmul(
                    dtp[:, j:j + 1], d_sb[:, j * P:(j + 1) * P], e0_f,
                    start=True, stop=True,
                )
            recip = epi.tile([P, 4], F32, name="recip")
            nc.vector.reciprocal(recip, dtp[:, 0:4])
            oT = epi.tile([P, TBW], BF16, name="oT")
            nc.vector.tensor_copy(oT, o_t)
            otp = mm_ps.tile([P, TBW], BF16, name="otp", tag="mm")
            for j in range(4):
                nc.tensor.transpose(
                    otp[:, j * P:(j + 1) * P],
                    oT[:, j * P:(j + 1) * P],
                    identq[0],
                )
            # two independent halves (DVE: j=0,1; Pool: j=2,3) so the
            # normalize chains run in parallel and each half's out-DMA can
            # start as soon as its own muls finish
            o_hlf = [epi.tile([P, 2, P], F32, name=f"o_out{h}")
                     for h in range(2)]
            for j in range(4):
                eng = nc.vector if j < 2 else _eng(nc, EPI_MUL_B)
                eng.tensor_scalar_mul(
                    out=o_hlf[j // 2][:, j % 2, :],
                    in0=otp[:, j * P:(j + 1) * P],
                    scalar1=recip[:, j:j + 1],
                )

            def do_dma():
                for h in range(2):
                    nc.scalar.dma_start(
                        out=out_d[tb * TBW + h * 2 * P:
                                  tb * TBW + (h + 1) * 2 * P, :].rearrange(
                            "(j p) h -> p j h", p=P),
                        in_=o_hlf[h],
                    )
            deferred.append([3, do_dma])

        def pop_pv():
            tb, si, off, es, first, last, o_t, d_t, dacc = pend.pop(0)
            if DENOM_MODE == "dve":
                # deferred to pop time so the DVE queue head never waits on
                # the exp/mask chain (es is certainly ready by now)
                if first:
                    nc.vector.tensor_copy(dacc, es)
                else:
                    nc.vector.tensor_add(dacc[:, off:], dacc[:, off:],
                                         es[:, off:])
            nc.tensor.matmul(o_t[:, off:], v_r[:, si, :], es[:, off:],
                             start=first, stop=last, skip_group_check=True)
            if DENOM_MODE == "pe":
                nc.tensor.matmul(d_t[:, off:], ones_bf, es[:, off:],
                                 start=first, stop=last,
                                 skip_group_check=True)
            if last:
                emit_epilogue(tb, o_t, d_t, dacc)

        # attention chunk state (carried across sections)
        attn_state = {}

        def emit_attn_chunk(tb, si):
            n_sc = (tb + 1) * (TBW // P)
            if si == 0:
                attn_state["o_t"] = o_ps.tile([P, TBW], F32, name="o_t", tag="o")
                attn_state["d_t"] = d_ps.tile([1, TBW], F32, name="d_t", tag="d")
                attn_state["dacc"] = (
                    dacc_pool.tile([P, TBW], BF16, name="dacc")
                    if DENOM_MODE == "dve" else None)
                attn_state["spair"] = None
            o_t, d_t, dacc = (attn_state["o_t"], attn_state["d_t"],
                              attn_state["dacc"])
            diag = si >= 4 * tb
            off = 0 if not diag else (si - 4 * tb) * P
            s_t = s_ps.tile([P, TBW], F32, name="s_t")
            nc.tensor.matmul(
                s_t[:, off:], kT[:, si * P:(si + 1) * P],
                qT[:, tb * TBW + off:(tb + 1) * TBW],
                start=True, stop=True,
            )
            es = es_pool.tile([P, TBW], BF16, name="es")
            nc.scalar.activation(out=es[:, off:], in_=s_t[:, off:],
                                 func=AF.Exp, scale=SCALE)
            if diag:
                # zero entries above the diagonal; only the leading 128
                # columns of the chunk straddle it
                nc.gpsimd.affine_select(
                    out=es[:, off:off + P], in_=es[:, off:off + P],
                    compare_op=mybir.AluOpType.is_ge,
                    fill=0.0, base=0,
                    pattern=[[1, P]], channel_multiplier=-1,
                )
            pend.append((tb, si, off, es, si == 0, si == n_sc - 1,
                         o_t, d_t, dacc))
            while len(pend) > PIPE:
                pop_pv()
            tick_deferred()

        def emit_x_dma(n):
            buf = x_g[n % 2]
            if XB in ("block", "block2"):
                # one 2MB DMA per block: amortizes the per-DMA completion
                # latency (the dominant fixed cost on HW) over 512 rows.
                # block2: alternate the SP HWDGE ring and the gpsimd SWDGE
                # ring so the two drain in parallel.
                eng = nc.sync if (XB == "block" or n % 2 == 0) else nc.gpsimd
                eng.dma_start(
                    out=buf,
                    in_=x_d[n * 4 * P:(n + 1) * 4 * P, :].rearrange(
                        "(tij p) e -> p tij e", p=P))
                return
            for tij in range(4):
                r0 = (n * 4 + tij) * P
                if XB == "tij2":
                    # alternate SP HWDGE ring / gpsimd SWDGE ring per tile
                    eng = nc.sync if tij % 2 == 0 else nc.gpsimd
                else:
                    eng = nc.sync if (XQ == 1 or tij % 2 == 0) else nc.scalar
                if n == 0:
                    # split block-0 loads so the first convert/transpose can
                    # start one half-tile (~0.7us) earlier
                    for hf in range(2):
                        cs = hf * (E // 2)
                        eng.dma_start(
                            out=buf[:, tij, cs:cs + E // 2],
                            in_=x_d[r0:r0 + P, cs:cs + E // 2])
                else:
                    eng.dma_start(out=buf[:, tij, :], in_=x_d[r0:r0 + P, :])

        def make_stageA(n):
            """Convert + transpose + projections + v-transpose emitters."""
            buf = x_g[n % 2]
            xbuf = xb[n % 2]
            emitters = []

            def conv_emitter(tij, half):
                # f32 -> bf16 ahead of the transpose: PE transposes then run
                # at 1 cyc/row (vs 2 for f32) and evacuations at 2x accel.
                def emit():
                    dst = xbuf[:, tij, half * 4 * P:(half + 1) * 4 * P]
                    src = buf[:, tij, half * 4 * P:(half + 1) * 4 * P]
                    k = (tij * 2 + half) % 2
                    if CONV_ENG == "pool":
                        nc.gpsimd.tensor_copy(dst, src)
                    elif CONV_ENG == "pool_dve":
                        (nc.gpsimd if k == 0 else nc.vector).tensor_copy(
                            dst, src)
                    elif CONV_ENG == "pool_act":
                        if k == 0:
                            nc.gpsimd.tensor_copy(dst, src)
                        else:
                            nc.scalar.copy(out=dst, in_=src)
                    elif CONV_ENG == "dve_act":
                        if k == 0:
                            nc.vector.tensor_copy(dst, src)
                        else:
                            nc.scalar.copy(out=dst, in_=src)
                    else:
                        nc.vector.tensor_copy(dst, src)
                return emit

            def tp_emitter(tij, half):
                def emit():
                    if TRANS == "f32":
                        tp = mm_ps.tile([P, 4 * P], F32, name="tp", tag="mm")
                        for j in range(4):
                            ec = 4 * half + j
                            nc.tensor.transpose(
                                tp[:, j * P:(j + 1) * P],
                                buf[:, tij, ec * P:(ec + 1) * P],
                                ident,
                            )
                    else:
                        tp = mm_ps.tile([P, 4 * P], BF16, name="tp", tag="mm")
                        for j in range(4):
                            ec = 4 * half + j
                            nc.tensor.transpose(
                                tp[:, j * P:(j + 1) * P],
                                xbuf[:, tij, ec * P:(ec + 1) * P],
                                identq[0],
                            )
                    dst = xT[:, 4 * half:4 * half + 4,
                             n * TBW + tij * P:n * TBW + (tij + 1) * P]
                    src = tp.rearrange("p (a b) -> p a b", b=P)
                    if tij % 2 == 0:
                        nc.vector.tensor_copy(dst, src)
                    elif TP_COPY_B == "act":
                        nc.scalar.copy(out=dst, in_=src)
                    else:
                        _eng(nc, TP_COPY_B).tensor_copy(dst, src)
                return emit

            for tij in range(4):
                for half in range(2):
                    if TRANS == "bf16":
                        emitters.append(conv_emitter(tij, half))
                    emitters.append(tp_emitter(tij, half))

            if stages == "xonly":
                return emitters

            vstage = vstage_pool.tile([P, TBW], BF16, name="vstage")

            def proj_emitter(w, dst_slice):
                def emit():
                    pt = mm_ps.tile([P, TBW], F32, name="pt", tag="mm")
                    for ec in range(NE):
                        nc.tensor.matmul(
                            pt, w[:, ec, :],
                            xT[:, ec, n * TBW:(n + 1) * TBW],
                            start=(ec == 0), stop=(ec == NE - 1),
                        )
                    if PROJ_EVAC == "act":
                        nc.scalar.copy(out=dst_slice, in_=pt)
                    else:
                        _eng(nc, PROJ_EVAC).tensor_copy(dst_slice, pt)
                return emit

            emitters.append(proj_emitter(wq_f, qT[:, n * TBW:(n + 1) * TBW]))
            emitters.append(proj_emitter(wk_f, kT[:, n * TBW:(n + 1) * TBW]))
            emitters.append(proj_emitter(wv_f, vstage))

            def vtrans_emitter():
                vp = mm_ps.tile([P, 4 * P], BF16, name="vp", tag="mm")
                ident_bf = identq[0]
                for j in range(4):
                    nc.tensor.transpose(
                        vp[:, j * P:(j + 1) * P],
                        vstage[:, j * P:(j + 1) * P],
                        ident_bf,
                    )
                nc.vector.tensor_copy(
                    v_r[:, n * 4:(n + 1) * 4, :].rearrange("p a b -> p (a b)"),
                    vp)

            emitters.append(vtrans_emitter)
            return emitters

        # ---- main interleaved schedule ----
        emit_x_dma(0)
        if emit_weights_cb is not None:
            w_f = emit_weights_cb()
        wq_f, wk_f, wv_f = w_f
        for n in range(NTB):
            if n + 1 < NTB:
                emit_x_dma(n + 1)
            stageA = make_stageA(n)
            if stages in ("xonly", "xproj") or n == 0:
                chunks = []
            else:
                tb = n - 1
                chunks = [(tb, si) for si in range((tb + 1) * 4)]
            ci = 0
            for i, emit in enumerate(stageA):
                emit()
                want = (i + 1) * len(chunks) // len(stageA)
                while ci < want:
                    emit_attn_chunk(*chunks[ci])
                    ci += 1
        if stages not in ("xonly", "xproj"):
            tb = NTB - 1
            for si in range((tb + 1) * 4):
                emit_attn_chunk(tb, si)
            while pend:
                pop_pv()
            tick_deferred(flush=True)


_CACHED = {}


def build_program(repeat: int = 1, stages: str = "all"):
    key = (repeat, stages)
    if key in _CACHED:
        return _CACHED[key]
    nc = bacc.Bacc("TRN2", target_bir_lowering=False, debug=False,
                   num_devices=B)
    x_d = nc.dram_tensor("x", [T, E],
                         BF16 if TRANS == "hostbf16" else F32,
                         kind="ExternalInput").ap()
    wq_d = nc.dram_tensor("Wq", [E, H], F32, kind="ExternalInput").ap()
    wk_d = nc.dram_tensor("Wk", [E, H], F32, kind="ExternalInput").ap()
    wv_d = nc.dram_tensor("Wv", [E, H], F32, kind="ExternalInput").ap()
    out_d = nc.dram_tensor("out", [T, H], F32, kind="ExternalOutput").ap()

    import contextlib
    with tile.TileContext(nc) as tc:
        with contextlib.ExitStack() as stack:
            persist = stack.enter_context(tc.tile_pool(name="persist", bufs=1))
            idents, ones_bf, e0_f, d_sb = emit_const_prologue(nc, tc, persist)
            pools = make_pools(tc, stack)
            xT = persist.tile([P, NE, T], BF16)     # [e_local, ec, t]
            qT = persist.tile([P, T], BF16)         # [h, t]
            kT = persist.tile([P, T], BF16)         # [h, s]
            v_r = persist.tile([P, NT, H], BF16)    # [s_local, sc, h]
            x_g = [pools["xstage"].tile(
                       [P, 4, E], BF16 if TRANS == "hostbf16" else F32,
                       name="x_g")
                   for _ in range(2)]
            xb = ([pools["xbf"].tile([P, 4, E], BF16, name="xb")
                   for _ in range(2)] if TRANS == "bf16" else x_g)
            res = (xT, qT, kT, v_r, x_g, xb)
            if repeat > 1:
                w_f = emit_weights(nc, persist, wq_d, wk_d, wv_d)
                with tc.For_i(0, repeat, 1):
                    emit_body(nc, tc, pools, idents, ones_bf, e0_f, d_sb,
                              w_f, res, x_d, out_d, stages=stages)
            else:
                emit_body(nc, tc, pools, idents, ones_bf, e0_f, d_sb,
                          None, res, x_d, out_d, stages=stages,
                          emit_weights_cb=lambda: emit_weights(
                              nc, persist, wq_d, wk_d, wv_d))
    nc.compile()
    _CACHED[key] = nc
    return nc


def prep_x(x):
    """Host-side cast of x to the kernel's DRAM dtype (bf16 halves the
    x DMA bytes; the kernel would round to bf16 on-chip anyway)."""
    if TRANS == "hostbf16":
        import ml_dtypes
        return np.ascontiguousarray(np.asarray(x).astype(ml_dtypes.bfloat16))
    return np.ascontiguousarray(np.asarray(x, dtype=np.float32))


def kernel(x, Wk, Wq, Wv):
    x = prep_x(x)
    Wk = np.ascontiguousarray(np.asarray(Wk, dtype=np.float32))
    Wq = np.ascontiguousarray(np.asarray(Wq, dtype=np.float32))
    Wv = np.ascontiguousarray(np.asarray(Wv, dtype=np.float32))
    assert x.shape == (B, T, E), x.shape

    nc = build_program()
    in_maps = [
        {"x": np.ascontiguousarray(x[c]), "Wq": Wq, "Wk": Wk, "Wv": Wv}
        for c in range(B)
    ]
    res = bass_utils.run_bass_kernel_spmd(nc, in_maps, core_ids=list(range(B)))
    return np.stack([res.results[c]["out"] for c in range(B)], axis=0)


if __name__ == "__main__":
    rng = np.random.default_rng(0)
    x = rng.standard_normal((B, T, E), dtype=np.float32)
    wq = (rng.standard_normal((E, H), dtype=np.float32) / np.sqrt(E)).astype(np.float32)
    wk = (rng.standard_normal((E, H), dtype=np.float32) / np.sqrt(E)).astype(np.float32)
    wv = (rng.standard_normal((E, H), dtype=np.float32) / np.sqrt(E)).astype(np.float32)
    out = kernel(x, wk, wq, wv)
    print("out", out.shape, out.dtype, float(np.abs(out).max()))



# revision 58
# speedup vs baseline: 3.0682x; 2.6876x over previous
"""Trainium2 Bass kernel for single-head causal attention (nn_Head).

Problem: x [B=8, T=2048, E=1024] f32; Wq/Wk/Wv [1024, 128] f32.
  q,k,v = x @ W*;  A = softmax(causal(q k^T / sqrt(H)));  out = A v.

Sharding: data-parallel over batch B — one batch element per NeuronCore
(8 cores), weights replicated. No collectives; outputs gathered host-side.

Per-core algorithm (T=2048, E=1024, H=128), v7:
  - x is converted f32->bf16 (DVE/ACT alternating) then PE-transposed in
    bf16 at 1 cyc/row; transpose PSUM evacuations run at 2x accel.
  - qT/kT/v/es are bf16: the attention matmuls (S = k^T q, O^T = v^T es)
    run at 1 cyc/row, the causal trapezoid is exact at 128 granularity,
    and the diagonal mask is one 128-wide affine_select per diag chunk.
  - Denominator: es accumulated into a bf16 dacc tile on DVE, one fold
    matmul per t-block, e0-matmul transpose trick, reciprocal on DVE.
  - Epilogue: PE-transpose of O^T, normalize into two independent halves
    (so the two out-DMAs can fire as soon as their half is scaled).
  - Emission interleaves attention of t-block n-1 with conv+transpose+
    proj of block n; block-0 x DMA is issued before the weight DMAs on
    the same SP queue (weights aren't needed until the first projection).
  - HW-measured breakdown (repeat-delta slope): x-load pipeline ~38us,
    +projections ~40us, +attention/epilogue ~6us. The projection stage is
    LDWEIGHTS-serialization-bound on HW (full-128-row stationaries are
    never pulled ahead by the PE reorder window).
"""

import numpy as np

import concourse.bass as bass
import concourse.mybir as mybir
import concourse.tile as tile
from concourse import bacc
from concourse import bass_utils
from concourse.masks import make_identity

F32 = mybir.dt.float32
F32R = mybir.dt.float32r
BF16 = mybir.dt.bfloat16
AF = mybir.ActivationFunctionType

B, T, E, H = 8, 2048, 1024, 128
P = 128                 # partitions
NE = E // P             # 8 e-chunks
NT = T // P             # 16 t-tiles
TBW = 512               # t-block width
NTB = T // TBW          # 4 t-blocks
SCALE = float(H) ** -0.5

DENOM_MODE = "dve"      # "pe" (ones-matmul per chunk) | "dve" (dacc on DVE)
import os as _os0
PIPE = int(_os0.environ.get("K_PIPE", "3"))  # PV trails S/exp by this many
import os as _os
PROJ_EVAC = _os.environ.get("K_PROJ_EVAC", "act")    # act | pool | dve
TP_COPY_B = _os.environ.get("K_TP_COPY_B", "act")    # engine for odd tij
EPI_MUL_B = _os.environ.get("K_EPI_MUL_B", "dve")    # engine for j=2,3 muls
CONV_ENG = _os.environ.get("K_CONV_ENG", "dve_act")  # pool|pool_dve|pool_act|dve|dve_act
TRANS = _os.environ.get("K_TRANS", "bf16")           # hostbf16 | bf16 | f32
XQ = int(_os.environ.get("K_XQ", "1"))               # x DMA queues (1|2)
XB = _os.environ.get("K_XB", "tij")                 # x DMA granularity
# TPERM: interleaved-t layout — partition p of col-group pair g holds DRAM
# rows {base+2p, base+2p+1} (8KB contiguous per partition -> 2x bigger DMA
# descriptors). All on-chip t-columns are then consistently permuted; only
# the causal mask (slope-2 affine) and the out-DMA row map change.
TPERM = _os.environ.get("K_TPERM", "0") == "1"


def _eng(nc, name):
    return {"act": None, "pool": nc.gpsimd, "dve": nc.vector}[name]


def emit_const_prologue(nc, tc, persist):
    ident = persist.tile([P, P], F32)
    make_identity(nc, ident)
    ident_bf = persist.tile([P, P], BF16)
    nc.vector.tensor_copy(ident_bf, ident)

    ones_f = persist.tile([P, 1], F32)
    nc.vector.memset(ones_f, 1.0)
    ones_bf = persist.tile([P, 1], BF16)
    nc.vector.tensor_copy(ones_bf, ones_f)
    # e0: unit vector selecting row 0; d_sb: staging for the denominator row
    # (rows 1..127 stay zero forever, so no per-iteration memset is needed)
    e0_f = persist.tile([P, 1], F32)
    nc.vector.memset(e0_f, 0.0)
    nc.vector.memset(e0_f[0:1, :], 1.0)
    d_sb = persist.tile([P, TBW], F32)
    nc.gpsimd.memset(d_sb, 0.0)
    return (ident, ident_bf), ones_bf, e0_f, d_sb


def emit_weights(nc, persist, wq_d, wk_d, wv_d):
    """Weight DMAs go on the same SP queue as x, so emit them AFTER block-0's
    x tiles: proj-q can't start before block-0 is transposed anyway, and x
    going first shrinks the startup PE-idle gap by ~6us."""
    w_f = []
    for name, wd in (("wq", wq_d), ("wk", wk_d), ("wv", wv_d)):
        wt = persist.tile([P, NE, H], F32, name=f"{name}_f")
        nc.sync.dma_start(out=wt, in_=wd.rearrange("(ec p) h -> p ec h", p=P))
        wb = persist.tile([P, NE, H], BF16, name=f"{name}_bf")
        nc.vector.tensor_copy(wb, wt)
        w_f.append(wb)
    return w_f


def make_pools(tc, stack):
    names = [("xstage", dict(bufs=2)),
             ("xbf", dict(bufs=2)),
             ("vstage", dict(bufs=2)),
             ("mm_ps", dict(bufs=3, space="PSUM")),
             ("s_ps", dict(bufs=3, space="PSUM")),
             ("o_ps", dict(bufs=1, space="PSUM")),
             ("d_ps", dict(bufs=1, space="PSUM")),
             ("es_pool", dict(bufs=6)),
             ("dacc_pool", dict(bufs=2)),
             ("epi", dict(bufs=2))]
    return {n: stack.enter_context(tc.tile_pool(name=n, **kw))
            for n, kw in names}


def emit_body(nc, tc, pools, idents, ones_bf, e0_f, d_sb, w_f, res,
              x_d, out_d, stages="all", emit_weights_cb=None):
    ident, identq = idents[0], (idents[1],)
    (xT, qT, kT, v_r, x_g, xb) = res
    vstage_pool = pools["vstage"]
    mm_ps, s_ps, o_ps, d_ps = (pools["mm_ps"], pools["s_ps"],
                               pools["o_ps"], pools["d_ps"])
    es_pool, dacc_pool, epi = (pools["es_pool"], pools["dacc_pool"],
                               pools["epi"])
    if True:

        pend = []
        deferred = []

        def tick_deferred(flush=False):
            for item in list(deferred):
                item[0] -= 1
                if flush or item[0] <= 0:
                    item[1]()
                    deferred.remove(item)

        def emit_epilogue(tb, o_t, d_t, dacc):
            if DENOM_MODE == "dve":
                nc.tensor.matmul(d_t, ones_bf, dacc, start=True, stop=True)
            # stage denominator row into d_sb row 0, transpose-replicate via
            # K=128 matmuls against e0 (rows 1.. of d_sb are always zero)
            nc.scalar.copy(out=d_sb[0:1, :], in_=d_t)
            dtp = mm_ps.tile([P, TBW], F32, name="dtp", tag="mm")
            for j in range(4):
                nc.tensor.matmul(
                    dtp[:, j:j + 1], d_sb[:, j * P:(j + 1) * P], e0_f,
                    start=True, stop=True,
                )
            recip = epi.tile([P, 4], F32, name="recip")
            nc.vector.reciprocal(recip, dtp[:, 0:4])
            oT = epi.tile([P, TBW], BF16, name="oT")
            nc.vector.tensor_copy(oT, o_t)
            otp = mm_ps.tile([P, TBW], BF16, name="otp", tag="mm")
            for j in range(4):
                nc.tensor.transpose(
                    otp[:, j * P:(j + 1) * P],
                    oT[:, j * P:(j + 1) * P],
                    identq[0],
                )
            # two independent halves (DVE: j=0,1; Pool: j=2,3) so the
            # normalize chains run in parallel and each half's out-DMA can
            # start as soon as its own muls finish
            o_hlf = [epi.tile([P, 2, P], F32, name=f"o_out{h}")
                     for h in range(2)]
            for j in range(4):
                eng = nc.vector if j < 2 else _eng(nc, EPI_MUL_B)
                eng.tensor_scalar_mul(
                    out=o_hlf[j // 2][:, j % 2, :],
                    in0=otp[:, j * P:(j + 1) * P],
                    scalar1=recip[:, j:j + 1],
                )

            def do_dma():
                for h in range(2):
                    dst = out_d[tb * TBW + h * 2 * P:
                                tb * TBW + (h + 1) * 2 * P, :]
                    if TPERM:
                        # partition p wrote t-rows {base + 2p + j}: two
                        # contiguous 512B rows per partition
                        dst = dst.rearrange("(p j) h -> p j h", j=2)
                    else:
                        dst = dst.rearrange("(j p) h -> p j h", p=P)
                    nc.scalar.dma_start(out=dst, in_=o_hlf[h])
            deferred.append([3, do_dma])

        def pop_pv():
            tb, si, off, es, first, last, o_t, d_t, dacc = pend.pop(0)
            if DENOM_MODE == "dve":
                # deferred to pop time so the DVE queue head never waits on
                # the exp/mask chain (es is certainly ready by now)
                if first:
                    nc.vector.tensor_copy(dacc, es)
                else:
                    nc.vector.tensor_add(dacc[:, off:], dacc[:, off:],
                                         es[:, off:])
            nc.tensor.matmul(o_t[:, off:], v_r[:, si, :], es[:, off:],
                             start=first, stop=last, skip_group_check=True)
            if DENOM_MODE == "pe":
                nc.tensor.matmul(d_t[:, off:], ones_bf, es[:, off:],
                                 start=first, stop=last,
                                 skip_group_check=True)
            if last:
                emit_epilogue(tb, o_t, d_t, dacc)

        # attention chunk state (carried across sections)
        attn_state = {}

        def emit_attn_chunk(tb, si):
            n_sc = (tb + 1) * (TBW // P)
            if si == 0:
                attn_state["o_t"] = o_ps.tile([P, TBW], F32, name="o_t", tag="o")
                attn_state["d_t"] = d_ps.tile([1, TBW], F32, name="d_t", tag="d")
                attn_state["dacc"] = (
                    dacc_pool.tile([P, TBW], BF16, name="dacc")
                    if DENOM_MODE == "dve" else None)
                attn_state["spair"] = None
            o_t, d_t, dacc = (attn_state["o_t"], attn_state["d_t"],
                              attn_state["dacc"])
            diag = si >= 4 * tb
            if TPERM:
                # col-groups hold interleaved t (t = Gbase + 2c + jt); the
                # trapezoid is 256-granular and the straddling 256-wide
                # t-pair needs two slope-2 affine masks
                rel = si - 4 * tb
                off = 0 if (not diag or rel < 2) else 2 * P
            else:
                off = 0 if not diag else (si - 4 * tb) * P
            s_t = s_ps.tile([P, TBW], F32, name="s_t")
            nc.tensor.matmul(
                s_t[:, off:], kT[:, si * P:(si + 1) * P],
                qT[:, tb * TBW + off:(tb + 1) * TBW],
                start=True, stop=True,
            )
            es = es_pool.tile([P, TBW], BF16, name="es")
            nc.scalar.activation(out=es[:, off:], in_=s_t[:, off:],
                                 func=AF.Exp, scale=SCALE)
            if diag and TPERM:
                js = si % 2
                for jt in range(2):
                    # keep where t - s = (2c + jt) - (2p + js) >= 0
                    nc.gpsimd.affine_select(
                        out=es[:, off + jt * P:off + (jt + 1) * P],
                        in_=es[:, off + jt * P:off + (jt + 1) * P],
                        compare_op=mybir.AluOpType.is_ge,
                        fill=0.0, base=jt - js,
                        pattern=[[2, P]], channel_multiplier=-2,
                    )
            elif diag:
                # zero entries above the diagonal; only the leading 128
                # columns of the chunk straddle it
                nc.gpsimd.affine_select(
                    out=es[:, off:off + P], in_=es[:, off:off + P],
                    compare_op=mybir.AluOpType.is_ge,
                    fill=0.0, base=0,
                    pattern=[[1, P]], channel_multiplier=-1,
                )
            pend.append((tb, si, off, es, si == 0, si == n_sc - 1,
                         o_t, d_t, dacc))
            while len(pend) > PIPE:
                pop_pv()
            tick_deferred()

        def emit_x_dma(n):
            buf = x_g[n % 2]
            if TPERM:
                # two 1MB DMAs per block; partition p of group-pair `pair`
                # gets rows {base + 2p, base + 2p + 1} = 8KB contiguous
                for pair in range(2):
                    base = n * 4 * P + pair * 2 * P
                    nc.sync.dma_start(
                        out=buf[:, 2 * pair:2 * pair + 2, :],
                        in_=x_d[base:base + 2 * P, :].rearrange(
                            "(p j) e -> p j e", j=2))
                return
            if XB in ("block", "block2"):
                # one 2MB DMA per block: amortizes the per-DMA completion
                # latency (the dominant fixed cost on HW) over 512 rows.
                # block2: alternate the SP HWDGE ring and the gpsimd SWDGE
                # ring so the two drain in parallel.
                eng = nc.sync if (XB == "block" or n % 2 == 0) else nc.gpsimd
                eng.dma_start(
                    out=buf,
                    in_=x_d[n * 4 * P:(n + 1) * 4 * P, :].rearrange(
                        "(tij p) e -> p tij e", p=P))
                return
            for tij in range(4):
                r0 = (n * 4 + tij) * P
                if XB == "tij2":
                    # alternate SP HWDGE ring / gpsimd SWDGE ring per tile
                    eng = nc.sync if tij % 2 == 0 else nc.gpsimd
                else:
                    eng = nc.sync if (XQ == 1 or tij % 2 == 0) else nc.scalar
                if n == 0:
                    # split block-0 loads so the first convert/transpose can
                    # start one half-tile (~0.7us) earlier
                    for hf in range(2):
                        cs = hf * (E // 2)
                        eng.dma_start(
                            out=buf[:, tij, cs:cs + E // 2],
                            in_=x_d[r0:r0 + P, cs:cs + E // 2])
                else:
                    eng.dma_start(out=buf[:, tij, :], in_=x_d[r0:r0 + P, :])

        def make_stageA(n):
            """Convert + transpose + projections + v-transpose emitters."""
            buf = x_g[n % 2]
            xbuf = xb[n % 2]
            emitters = []

            def conv_emitter(tij, half):
                # f32 -> bf16 ahead of the transpose: PE transposes then run
                # at 1 cyc/row (vs 2 for f32) and evacuations at 2x accel.
                def emit():
                    dst = xbuf[:, tij, half * 4 * P:(half + 1) * 4 * P]
                    src = buf[:, tij, half * 4 * P:(half + 1) * 4 * P]
                    k = (tij * 2 + half) % 2
                    if CONV_ENG == "pool":
                        nc.gpsimd.tensor_copy(dst, src)
                    elif CONV_ENG == "pool_dve":
                        (nc.gpsimd if k == 0 else nc.vector).tensor_copy(
                            dst, src)
                    elif CONV_ENG == "pool_act":
                        if k == 0:
                            nc.gpsimd.tensor_copy(dst, src)
                        else:
                            nc.scalar.copy(out=dst, in_=src)
                    elif CONV_ENG == "dve_act":
                        if k == 0:
                            nc.vector.tensor_copy(dst, src)
                        else:
                            nc.scalar.copy(out=dst, in_=src)
                    else:
                        nc.vector.tensor_copy(dst, src)
                return emit

            def tp_emitter(tij, half):
                def emit():
                    if TRANS == "f32":
                        tp = mm_ps.tile([P, 4 * P], F32, name="tp", tag="mm")
                        for j in range(4):
                            ec = 4 * half + j
                            nc.tensor.transpose(
                                tp[:, j * P:(j + 1) * P],
                                buf[:, tij, ec * P:(ec + 1) * P],
                                ident,
                            )
                    else:
                        tp = mm_ps.tile([P, 4 * P], BF16, name="tp", tag="mm")
                        for j in range(4):
                            ec = 4 * half + j
                            nc.tensor.transpose(
                                tp[:, j * P:(j + 1) * P],
                                xbuf[:, tij, ec * P:(ec + 1) * P],
                                identq[0],
                            )
                    dst = xT[:, 4 * half:4 * half + 4,
                             n * TBW + tij * P:n * TBW + (tij + 1) * P]
                    src = tp.rearrange("p (a b) -> p a b", b=P)
                    if tij % 2 == 0:
                        nc.vector.tensor_copy(dst, src)
                    elif TP_COPY_B == "act":
                        nc.scalar.copy(out=dst, in_=src)
                    else:
                        _eng(nc, TP_COPY_B).tensor_copy(dst, src)
                return emit

            for tij in range(4):
                for half in range(2):
                    if TRANS == "bf16":
                        emitters.append(conv_emitter(tij, half))
                    emitters.append(tp_emitter(tij, half))

            if stages == "xonly":
                return emitters

            vstage = vstage_pool.tile([P, TBW], BF16, name="vstage")

            def proj_emitter(w, dst_slice):
                def emit():
                    pt = mm_ps.tile([P, TBW], F32, name="pt", tag="mm")
                    for ec in range(NE):
                        nc.tensor.matmul(
                            pt, w[:, ec, :],
                            xT[:, ec, n * TBW:(n + 1) * TBW],
                            start=(ec == 0), stop=(ec == NE - 1),
                        )
                    if PROJ_EVAC == "act":
                        nc.scalar.copy(out=dst_slice, in_=pt)
                    else:
                        _eng(nc, PROJ_EVAC).tensor_copy(dst_slice, pt)
                return emit

            emitters.append(proj_emitter(wq_f, qT[:, n * TBW:(n + 1) * TBW]))
            emitters.append(proj_emitter(wk_f, kT[:, n * TBW:(n + 1) * TBW]))
            emitters.append(proj_emitter(wv_f, vstage))

            def vtrans_emitter():
                vp = mm_ps.tile([P, 4 * P], BF16, name="vp", tag="mm")
                ident_bf = identq[0]
                for j in range(4):
                    nc.tensor.transpose(
                        vp[:, j * P:(j + 1) * P],
                        vstage[:, j * P:(j + 1) * P],
                        ident_bf,
                    )
                nc.vector.tensor_copy(
                    v_r[:, n * 4:(n + 1) * 4, :].rearrange("p a b -> p (a b)"),
                    vp)

            emitters.append(vtrans_emitter)
            return emitters

        # ---- main interleaved schedule ----
        emit_x_dma(0)
        if emit_weights_cb is not None:
            w_f = emit_weights_cb()
        wq_f, wk_f, wv_f = w_f
        for n in range(NTB):
            if n + 1 < NTB:
                emit_x_dma(n + 1)
            stageA = make_stageA(n)
            if stages in ("xonly", "xproj") or n == 0:
                chunks = []
            else:
                tb = n - 1
                chunks = [(tb, si) for si in range((tb + 1) * 4)]
            ci = 0
            for i, emit in enumerate(stageA):
                emit()
                want = (i + 1) * len(chunks) // len(stageA)
                while ci < want:
                    emit_attn_chunk(*chunks[ci])
                    ci += 1
        if stages not in ("xonly", "xproj"):
            tb = NTB - 1
            for si in range((tb + 1) * 4):
                emit_attn_chunk(tb, si)
            while pend:
                pop_pv()
            tick_deferred(flush=True)


_CACHED = {}


def build_program(repeat: int = 1, stages: str = "all"):
    key = (repeat, stages)
    if key in _CACHED:
        return _CACHED[key]
    nc = bacc.Bacc("TRN2", target_bir_lowering=False, debug=False,
                   num_devices=B)
    x_d = nc.dram_tensor("x", [T, E],
                         BF16 if TRANS == "hostbf16" else F32,
                         kind="ExternalInput").ap()
    wq_d = nc.dram_tensor("Wq", [E, H], F32, kind="ExternalInput").ap()
    wk_d = nc.dram_tensor("Wk", [E, H], F32, kind="ExternalInput").ap()
    wv_d = nc.dram_tensor("Wv", [E, H], F32, kind="ExternalInput").ap()
    out_d = nc.dram_tensor("out", [T, H], F32, kind="ExternalOutput").ap()

    import contextlib
    with tile.TileContext(nc) as tc:
        with contextlib.ExitStack() as stack:
            persist = stack.enter_context(tc.tile_pool(name="persist", bufs=1))
            idents, ones_bf, e0_f, d_sb = emit_const_prologue(nc, tc, persist)
            pools = make_pools(tc, stack)
            xT = persist.tile([P, NE, T], BF16)     # [e_local, ec, t]
            qT = persist.tile([P, T], BF16)         # [h, t]
            kT = persist.tile([P, T], BF16)         # [h, s]
            v_r = persist.tile([P, NT, H], BF16)    # [s_local, sc, h]
            x_g = [pools["xstage"].tile(
                       [P, 4, E], BF16 if TRANS == "hostbf16" else F32,
                       name="x_g")
                   for _ in range(2)]
            xb = ([pools["xbf"].tile([P, 4, E], BF16, name="xb")
                   for _ in range(2)] if TRANS == "bf16" else x_g)
            res = (xT, qT, kT, v_r, x_g, xb)
            if repeat > 1:
                w_f = emit_weights(nc, persist, wq_d, wk_d, wv_d)
                with tc.For_i(0, repeat, 1):
                    emit_body(nc, tc, pools, idents, ones_bf, e0_f, d_sb,
                              w_f, res, x_d, out_d, stages=stages)
            else:
                emit_body(nc, tc, pools, idents, ones_bf, e0_f, d_sb,
                          None, res, x_d, out_d, stages=stages,
                          emit_weights_cb=lambda: emit_weights(
                              nc, persist, wq_d, wk_d, wv_d))
    nc.compile()
    _CACHED[key] = nc
    return nc


def prep_x(x):
    """Host-side cast of x to the kernel's DRAM dtype (bf16 halves the
    x DMA bytes; the kernel would round to bf16 on-chip anyway)."""
    if TRANS == "hostbf16":
        import ml_dtypes
        return np.ascontiguousarray(np.asarray(x).astype(ml_dtypes.bfloat16))
    return np.ascontiguousarray(np.asarray(x, dtype=np.float32))


def kernel(x, Wk, Wq, Wv):
    x = prep_x(x)
    Wk = np.ascontiguousarray(np.asarray(Wk, dtype=np.float32))
    Wq = np.ascontiguousarray(np.asarray(Wq, dtype=np.float32))
    Wv = np.ascontiguousarray(np.asarray(Wv, dtype=np.float32))
    assert x.shape == (B, T, E), x.shape

    nc = build_program()
    in_maps = [
        {"x": np.ascontiguousarray(x[c]), "Wq": Wq, "Wk": Wk, "Wv": Wv}
        for c in range(B)
    ]
    res = bass_utils.run_bass_kernel_spmd(nc, in_maps, core_ids=list(range(B)))
    return np.stack([res.results[c]["out"] for c in range(B)], axis=0)


if __name__ == "__main__":
    rng = np.random.default_rng(0)
    x = rng.standard_normal((B, T, E), dtype=np.float32)
    wq = (rng.standard_normal((E, H), dtype=np.float32) / np.sqrt(E)).astype(np.float32)
    wk = (rng.standard_normal((E, H), dtype=np.float32) / np.sqrt(E)).astype(np.float32)
    wv = (rng.standard_normal((E, H), dtype=np.float32) / np.sqrt(E)).astype(np.float32)
    out = kernel(x, wk, wq, wv)
    print("out", out.shape, out.dtype, float(np.abs(out).max()))

